# revision 10
# baseline (speedup 1.0000x reference)
"""HONU (order-2, L=64) forward as a per-row quadratic form on 8 trn2 cores.

Reference: out[i] = sum_{j<=k} W[p(j,k)] x[i,j] x[i,k] + b = x_i^T A x_i + b
with A upper-triangular scattered from W.  Pure data parallel over the batch.

Per-core program (SHARD=2048 rows), ~24 instructions, bf16 matmul path
(host-simulated rel err ~3e-3, gate is 2e-2):

  * x is DMA'd in ONE gpsimd (SWDGE) transfer that casts fp32->bf16 inline,
    with 2KB-contiguous DRAM chunks per partition: partition p holds rows
    {cb*1024 + 8p + t : cb in 2, t in 8} (row-permuted layout for
    descriptor efficiency; the baseline's 256B chunks were DMA-packet-rate
    bound at ~80GB/s).
  * 8 PE transposes (bf16, 1 cyc/row) turn each [128 rows, (2cb x 64feat)]
    slice into the packed layout xt[cb*64+m, p] (two row-blocks stacked on
    partitions, blockdiag trick).
  * 2 matmuls with blockdiag(A,A) (bf16, N=512) give yt; DVE mul z = xt*yt;
    2 matmuls with the block-ones matrix reduce the feature partitions.
  * Bias-add (+ the row un-permutation, free in this op's access patterns)
    so the final out DMA is 2 fat contiguous descriptors.
"""

import math
from contextlib import ExitStack
from itertools import combinations_with_replacement

import numpy as np

import concourse.bacc as bacc
import concourse.bass as bass
import concourse.tile as tile
from concourse import mybir
from concourse.bass_utils import run_bass_kernel_spmd

L = 64
ORDER = 2
B = 16384
N_CORES = 8
SHARD = B // N_CORES  # 2048
HALF = SHARD // 2  # 1024
NUM_W = math.comb(L + 1 + ORDER - 1, ORDER)  # 2145 (only first 2080 used)

IDX = np.array(list(combinations_with_replacement(range(L), ORDER)), dtype=np.int32)

F32 = mybir.dt.float32
BF16 = mybir.dt.bfloat16

_program_cache = {}


def _build_program(bias: float, compile: bool = True) -> bass.Bass:
    nc = bacc.Bacc()

    x_in = nc.declare_dram_parameter("x", [SHARD, L], F32, isOutput=False)
    cons_in = nc.declare_dram_parameter("cons", [128, 258], BF16, isOutput=False)
    out_t = nc.declare_dram_parameter("out", [SHARD, 1], F32, isOutput=True)

    # Row permutation: partition p, half cb, slot t  <->  row cb*1024 + 8p + t.
    # Per (p, cb) the 8 rows are contiguous in DRAM -> 2KB descriptor chunks.
    xv = x_in[:, :].rearrange("(cb p t) m -> p cb t m", cb=2, p=128, t=8)
    # Output rows: partition 0 -> rows 0..1023, partition 1 -> rows 1024..2047.
    out_v = out_t[:, :].rearrange("(cb r) one -> cb (r one)", cb=2)

    with ExitStack() as ctx:
        tc = ctx.enter_context(tile.TileContext(nc))
        consts = ctx.enter_context(tc.tile_pool(name="consts", bufs=1))
        xin_pool = ctx.enter_context(tc.tile_pool(name="xin", bufs=1))
        xt_pool = ctx.enter_context(tc.tile_pool(name="xt", bufs=1))
        z_pool = ctx.enter_context(tc.tile_pool(name="z", bufs=1))
        out_pool = ctx.enter_context(tc.tile_pool(name="outp", bufs=1))
        warm_ps = ctx.enter_context(tc.tile_pool(name="warm", bufs=1, space="PSUM"))
        ps_xt = ctx.enter_context(tc.tile_pool(name="ps_xt", bufs=1, space="PSUM"))
        ps_yt = ctx.enter_context(tc.tile_pool(name="ps_yt", bufs=1, space="PSUM"))
        ps_o = ctx.enter_context(tc.tile_pool(name="ps_o", bufs=1, space="PSUM"))

        cons = consts.tile([128, 258], BF16)
        nc.scalar.dma_start(out=cons[:], in_=cons_in[:, :])

        # SWDGE casting DMA: fp32 in DRAM -> bf16 in SBUF, one transfer.
        # Storage layout [p, t, cb, m] so each transpose input slice is one
        # contiguous [128, 128] block; the DMA write side is permuted to
        # match the DRAM iteration order (cb, t, m).
        xb = xin_pool.tile([128, 8, 2, L], BF16)
        nc.gpsimd.dma_start(out=xb[:, :, 0, :], in_=xv[:, 0])
        nc.gpsimd.dma_start(out=xb[:, :, 1, :], in_=xv[:, 1])

        a2 = cons[:, 0:128]
        eye = cons[:, 128:256]
        ew = cons[:, 256:258]

        # PE warmup: touch the consts tile once on the PE so later matmuls
        # carry at most one sync wait (walrus rejects Matmult with >1 wait).
        w1 = warm_ps.tile([128, 1], F32)
        nc.tensor.matmul(w1[:, 0:1], lhsT=eye, rhs=a2[:, 0:1], start=True, stop=True)

        # 8 PE transposes -> packed xt in PSUM ([128, 1024] bf16 = 1 bank).
        pxt = ps_xt.tile([128, 1024], BF16)
        for t in range(8):
            col = t * 128
            nc.tensor.transpose(
                pxt[:, col : col + 128],
                xb[:, t, :, :].rearrange("p cb m -> p (cb m)"),
                eye,
            )

        xt = xt_pool.tile([128, 2, 512], BF16)
        nc.scalar.activation(
            xt[:, 0, :], pxt[:, 0:512], mybir.ActivationFunctionType.Copy
        )
        nc.scalar.activation(
            xt[:, 1, :], pxt[:, 512:1024], mybir.ActivationFunctionType.Copy
        )

        # yt = blockdiag(A,A)^T @ xt   (bf16, N=512)
        pyt = ps_yt.tile([128, 1024], F32)
        nc.tensor.matmul(
            pyt[:, 0:512], lhsT=a2, rhs=xt[:, 0, :], start=True, stop=True
        )
        nc.tensor.matmul(
            pyt[:, 512:1024], lhsT=a2, rhs=xt[:, 1, :], start=True, stop=True
        )

        # z = xt * yt  (bf16 out; in1 reads PSUM fp32)
        z = z_pool.tile([128, 2, 512], BF16)
        nc.vector.tensor_mul(z[:, 0, :], xt[:, 0, :], pyt[:, 0:512])
        nc.vector.tensor_mul(z[:, 1, :], xt[:, 1, :], pyt[:, 512:1024])

        # po[cb, t*128 + p] = out_row(cb*1024 + 8p + t) - b
        po = ps_o.tile([2, 1024], F32)
        nc.tensor.matmul(po[:, 0:512], lhsT=ew, rhs=z[:, 0, :], start=True, stop=True)
        nc.tensor.matmul(
            po[:, 512:1024], lhsT=ew, rhs=z[:, 1, :], start=True, stop=True
        )

        # Bias-add + undo the row permutation: out_sb[cb, 8p+t] = po[cb, t*128+p] + b
        out_sb = out_pool.tile([2, HALF], F32)
        out_vw = out_sb[:, :].rearrange("cb (p t) -> cb t p", p=128, t=8)
        po_vw = po[:, :].rearrange("cb (t p) -> cb t p", t=8, p=128)
        nc.vector.tensor_scalar_add(out_vw[:, 0:4, :], po_vw[:, 0:4, :], bias)
        nc.vector.tensor_scalar_add(out_vw[:, 4:8, :], po_vw[:, 4:8, :], bias)

        nc.sync.dma_start(out=out_v, in_=out_sb[:])

    if compile:
        nc.compile()
    return nc


def _get_program(bias: float) -> bass.Bass:
    key = float(bias)
    if key not in _program_cache:
        _program_cache[key] = _build_program(key)
    return _program_cache[key]


def _host_constants(W: np.ndarray):
    from ml_dtypes import bfloat16

    A = np.zeros((L, L), dtype=np.float32)
    A[IDX[:, 0], IDX[:, 1]] = W[: IDX.shape[0]].astype(np.float32)
    C = np.zeros((128, 258), dtype=np.float32)
    C[:64, 0:64] = A
    C[64:, 64:128] = A
    C[:, 128:256] = np.eye(128, dtype=np.float32)
    C[:64, 256] = 1.0
    C[64:, 257] = 1.0
    return C.astype(bfloat16)


def _run(x, W, b, trace=False):
    x = np.ascontiguousarray(np.asarray(x, dtype=np.float32))
    W = np.asarray(W, dtype=np.float32)
    b = np.asarray(b, dtype=np.float32)
    assert x.shape == (B, L), x.shape

    C = _host_constants(W)
    nc = _get_program(float(b.reshape(-1)[0]))
    in_maps = [
        {"x": x[c * SHARD : (c + 1) * SHARD], "cons": C}
        for c in range(N_CORES)
    ]
    res = run_bass_kernel_spmd(nc, in_maps, core_ids=list(range(N_CORES)), trace=trace)
    out = np.concatenate([res.results[c]["out"] for c in range(N_CORES)], axis=0)
    return out, res


def kernel(x, W, b):
    out, _ = _run(x, W, b)
    return out
